# revision 42
# baseline (speedup 1.0000x reference)
"""CLIPAttention (B=8, S=1024, D=1024, H=16) Trainium2 Bass kernel.

Strategy: data-parallel over batch — one batch element per NeuronCore (8 cores).
Per core, the whole attention layer runs on-chip:

  - Host pre-transposes/casts weights + hidden states to bf16 (fp32 PSUM accum).
  - qT/kT projections produce Q^T/K^T in [d, s] layout (d on partitions), so
    scores are computed *transposed*: scoresT[k, q] with lhsT = kT (stationary),
    rhs = qT (moving). exp() runs on the ACT engine straight out of PSUM (no
    max subtraction needed: |scores| is O(6) here, exp is safe in fp32).
  - Causality is structural: score tiles with k_block > q are never computed;
    the diagonal 128x128 block gets a triangular -1e30 mask added in PSUM.
  - Head pairs (2c, 2c+1) live on partition halves 0-63 / 64-127 of d-chunk c,
    so their K=64 score matmuls land on disjoint PE row groups and overlap.
  - P@V needs no probs transpose: ctxT[d, q] = sum_k v[k, d] * expT[k, q] with
    lhsT = v (natural [s, d] layout, with a ones-column appended per head so
    the same matmul emits the softmax denominator row), rhs = expT.
  - Per-(head, q) normalization: reciprocal of the denominator row, broadcast
    across 64 partitions on the otherwise-idle GPSIMD, applied with a DVE
    multiply. Normalizing ctx (not probs) keeps the work O(S*HD) per head.
  - Out-projection contracts d (all heads) from ctxT directly; bv/bo biases
    enter via ones-row K=1 matmuls (bq/bk are per-partition DVE biases).
  - Software pipeline per head-pair iteration: scores(c) [+ P@V(c-1)
    interleaved between score blocks] then the qT/kT projection for chunk
    c+1 — so the ACT exp stream starts early and runs under the PE-heavy
    projection work.

All shapes/strides hardcoded for this problem.
"""

import numpy as np
import ml_dtypes

import concourse.bass as bass
import concourse.mybir as mybir
import concourse.tile as tile
from concourse import bacc
import concourse.bass_utils as bass_utils

B, S, D, H = 8, 1024, 1024, 16
HD = D // H
SCALE = HD ** -0.5
P = 128
NCH = D // P  # 8 chunks of 128
N_CORES = 8

F32 = mybir.dt.float32
BF16 = mybir.dt.bfloat16
EXP = mybir.ActivationFunctionType.Exp
bf16 = ml_dtypes.bfloat16

NEG = -1.0e30
# scores PSUM granularity: True = [128,1024] x3 slots, one exp per (head, kb);
# False = [128,<=512] x6 slots, exp per q-tile piece
SC_WIDE = False


def build_bass(loop_n=None, with_bias=True):
    nc = bacc.Bacc(
        "TRN2",
        target_bir_lowering=False,
        debug=False,
        enable_asserts=False,
        num_devices=N_CORES,
    )

    hsT_d = nc.dram_tensor("hsT", [D, S], BF16, kind="ExternalInput").ap()
    wq_d = nc.dram_tensor("wqT", [D, D], BF16, kind="ExternalInput").ap()
    wk_d = nc.dram_tensor("wkT", [D, D], BF16, kind="ExternalInput").ap()
    wv_d = nc.dram_tensor("wvT", [D, D], BF16, kind="ExternalInput").ap()
    wo_d = nc.dram_tensor("woT", [D, D], BF16, kind="ExternalInput").ap()
    bq_d = nc.dram_tensor("bqc", [P, NCH], F32, kind="ExternalInput").ap()
    bk_d = nc.dram_tensor("bkc", [P, NCH], F32, kind="ExternalInput").ap()
    bv_d = nc.dram_tensor("bvr", [1, D], BF16, kind="ExternalInput").ap()
    bo_d = nc.dram_tensor("bor", [1, D], BF16, kind="ExternalInput").ap()
    mask_d = nc.dram_tensor("maskT", [P, P], BF16, kind="ExternalInput").ap()
    out_d = nc.dram_tensor("out", [S, D], F32, kind="ExternalOutput").ap()

    aps = (hsT_d, wq_d, wk_d, wv_d, wo_d, bq_d, bk_d, bv_d, bo_d, mask_d, out_d)
    with tile.TileContext(nc) as tc:
        pools = (
            tc.alloc_tile_pool(name="persist", bufs=1),
            tc.alloc_tile_pool(name="epool", bufs=2),
            tc.alloc_tile_pool(name="scratch", bufs=2),
            tc.alloc_tile_pool(name="psum", bufs=2, space="PSUM"),
        )
        if loop_n is None:
            _kernel_body(tc, pools, with_bias, *aps)
        else:
            hints = (
                mybir.EngineType.PE,
                mybir.EngineType.Activation,
                mybir.EngineType.DVE,
                mybir.EngineType.SP,
            )
            with tc.For_i(0, loop_n, 1, hint_engines=hints):
                _kernel_body(tc, pools, with_bias, *aps)
        for pool in reversed(pools):
            pool.release()
    nc.compile()
    return nc


def _kernel_body(tc, pools, with_bias, hsT_d, wq_d, wk_d, wv_d, wo_d,
                 bq_d, bk_d, bv_d, bo_d, mask_d, out_d):
    nc = tc.nc
    persist, epool, scratch, psum = pools

    # ---- persistent SBUF tensors -------------------------------------------
    hsT_sb = persist.tile([P, NCH, S], BF16, name="hsT_sb")
    wq_sb = persist.tile([P, NCH, D], BF16, name="wq_sb")
    wk_sb = persist.tile([P, NCH, D], BF16, name="wk_sb")
    wv_sb = persist.tile([P, NCH, D], BF16, name="wv_sb")
    wo_sb = persist.tile([P, NCH, D], BF16, name="wo_sb")
    v_sb = persist.tile([P, NCH, H, HD + 1], BF16, name="v_sb")
    ctxT_sb = persist.tile([P, NCH, S], BF16, name="ctxT_sb")
    mask_sb = persist.tile([P, P], BF16, name="mask_sb")
    ones_sb = persist.tile([1, P], BF16, name="ones_sb")
    if with_bias:
        bq_sb = persist.tile([P, NCH], F32, name="bq_sb")
        bk_sb = persist.tile([P, NCH], F32, name="bk_sb")
        bv_sb = persist.tile([1, D], BF16, name="bv_sb")
        bo_sb = persist.tile([1, D], BF16, name="bo_sb")

    # ---- input DMAs (per-chunk so compute can start early) ------------------
    hsT_r = hsT_d.rearrange("(c p) s -> c p s", p=P)
    wq_r = wq_d.rearrange("(c p) n -> c p n", p=P)
    wk_r = wk_d.rearrange("(c p) n -> c p n", p=P)
    wv_r = wv_d.rearrange("(c p) n -> c p n", p=P)
    wo_r = wo_d.rearrange("(c p) n -> c p n", p=P)
    # qT proj consumes (hsT[k], wq[k]) pairs in k order — interleave those
    # DMAs so the PE can start ~1us in and stream behind the DMA engines
    for c in range(NCH):
        nc.sync.dma_start(out=hsT_sb[:, c, :], in_=hsT_r[c])
        nc.sync.dma_start(out=wq_sb[:, c, :], in_=wq_r[c])
    for c in range(NCH):
        nc.sync.dma_start(out=wk_sb[:, c, :], in_=wk_r[c])
    for c in range(NCH):
        nc.sync.dma_start(out=wv_sb[:, c, :], in_=wv_r[c])
    for c in range(NCH):
        nc.sync.dma_start(out=wo_sb[:, c, :], in_=wo_r[c])
    if with_bias:
        nc.sync.dma_start(out=bq_sb, in_=bq_d)
        nc.sync.dma_start(out=bk_sb, in_=bk_d)
        nc.sync.dma_start(out=bv_sb, in_=bv_d)
        nc.sync.dma_start(out=bo_sb, in_=bo_d)
    nc.sync.dma_start(out=mask_sb, in_=mask_d)
    nc.vector.memset(ones_sb, 1.0)
    # ones columns for the denominator trick: memset everything, the V
    # projection copies then overwrite the data columns
    nc.vector.memset(v_sb.rearrange("p c h e -> p (c h e)"), 1.0)

    # ---- qT/kT projection for chunk c (heads 2c, 2c+1) ----------------------
    def emit_qk_proj(c):
        outs = []
        for w_sb, bias, nm in (
            (wq_sb, bq_sb[:, c : c + 1] if with_bias else None, "q"),
            (wk_sb, bk_sb[:, c : c + 1] if with_bias else None, "k"),
        ):
            o_sb = scratch.tile([P, S], BF16, tag=f"{nm}T", name=f"{nm}T_{c}", bufs=2)
            for st in range(2):
                ps = psum.tile([P, 512], F32, tag="ctx", name=f"{nm}ps_{c}_{st}", bufs=5)
                for k in range(NCH):
                    nc.tensor.matmul(
                        ps,
                        lhsT=w_sb[:, k, c * P : (c + 1) * P],
                        rhs=hsT_sb[:, k, st * 512 : (st + 1) * 512],
                        start=(k == 0),
                        stop=(k == NCH - 1),
                    )
                dst = o_sb[:, st * 512 : (st + 1) * 512]
                if with_bias:
                    nc.vector.tensor_scalar_add(dst, ps, bias)
                else:
                    nc.vector.tensor_copy(dst, ps)
            outs.append(o_sb)
        return outs

    # ---- V projection: v[s, d] = hs @ Wv.T + bv  (natural layout) -----------
    def emit_v_pair(m):  # s chunk m, both 512-wide d tiles (8 heads each)
        for nt in range(2):
            ps = psum.tile([P, 512], F32, tag="ctx", name=f"vps_{m}_{nt}", bufs=5)
            for c in range(NCH):
                nc.tensor.matmul(
                    ps,
                    lhsT=hsT_sb[:, c, m * P : (m + 1) * P],
                    rhs=wv_sb[:, c, nt * 512 : (nt + 1) * 512],
                    start=(c == 0),
                    stop=(not with_bias) and (c == NCH - 1),
                )
            if with_bias:
                nc.tensor.matmul(
                    ps,
                    lhsT=ones_sb[:, 0:P],
                    rhs=bv_sb[:, nt * 512 : (nt + 1) * 512],
                    start=False,
                    stop=True,
                )
            nc.scalar.copy(
                v_sb[:, m, 8 * nt : 8 * (nt + 1), 0:HD],
                ps.rearrange("p (h e) -> p h e", h=8),
            )

    # ---- scoresT[k, q] + exp for heads 2c, 2c+1 -----------------------------
    # Interleaved so consecutive matmuls sit on disjoint PE row groups
    # (rows 0-63 vs 64-127) and overlap. E[hh][kb] is [128, 1024 - kb*128]
    # covering q in [kb*128, 1024). Emitted as 8 closures (one per kb) so
    # P@V work can be woven between them.
    def scores_unit(c, kb, qT, kT, E_pair):
        lo = kb * P
        spans = [(lo, 512), (512, 1024)] if lo < 512 else [(lo, 1024)]
        E_tiles = []
        for hh in range(2):
            h = 2 * c + hh
            E_tiles.append(
                epool.tile([P, S - lo], BF16, tag=f"E{kb}_{hh}",
                           name=f"E_{h}_{kb}", bufs=3 if kb >= 4 else 2)
            )
            E_pair[hh].append(E_tiles[hh])
        if SC_WIDE:
            pss = [
                psum.tile([P, 1024], F32, tag="sc", name=f"sps_{2*c+hh}_{kb}", bufs=3)
                for hh in range(2)
            ]
            for a, b in spans:
                for hh in range(2):
                    po = hh * 64
                    nc.tensor.matmul(
                        pss[hh][:, a:b],
                        lhsT=kT[po : po + 64, lo : lo + P],
                        rhs=qT[po : po + 64, a:b],
                        start=True,
                        stop=True,
                    )
            for hh in range(2):
                nc.scalar.activation(E_tiles[hh], pss[hh][:, lo:1024], EXP)
                # exp(s + mask) == exp(s) * (mask == 0): zero the upper
                # triangle of the diagonal block (bf16 2x DVE mode)
                nc.vector.tensor_mul(
                    E_tiles[hh][:, 0:P], E_tiles[hh][:, 0:P], mask_sb
                )
            return
        for a, b in spans:
            pss = [
                psum.tile(
                    [P, b - a], F32, tag="sc", name=f"sps_{2*c+hh}_{kb}_{a}", bufs=3
                )
                for hh in range(2)
            ]
            for hh in range(2):
                po = hh * 64
                nc.tensor.matmul(
                    pss[hh],
                    lhsT=kT[po : po + 64, lo : lo + P],
                    rhs=qT[po : po + 64, a:b],
                    start=True,
                    stop=True,
                )
            for hh in range(2):
                nc.scalar.activation(E_tiles[hh][:, a - lo : b - lo], pss[hh], EXP)
                if a == lo:
                    # exp(s + mask) == exp(s) * (mask == 0): zero the upper
                    # triangle of the diagonal block (bf16 2x DVE mode)
                    nc.vector.tensor_mul(
                        E_tiles[hh][:, 0:P], E_tiles[hh][:, 0:P], mask_sb
                    )

    # ---- P@V + normalization for one (head, q-tile) -------------------------
    def pv_unit(c, hh, qt, E_pair):
        h = 2 * c + hh
        po = hh * 64
        E = E_pair[hh]
        cps = psum.tile([HD + 1, 512], F32, tag="ctx", name=f"cps_{h}_{qt}", bufs=5)
        kmax = 3 if qt == 0 else 7
        for kb in range(kmax + 1):
            lo = kb * P
            off = max(0, lo - qt * 512)
            nc.tensor.matmul(
                cps[:, off:512],
                lhsT=v_sb[:, kb, h, :],
                rhs=E[kb][:, qt * 512 + off - lo : (qt + 1) * 512 - lo],
                start=(kb == 0),
                stop=(kb == kmax),
            )
        rec = scratch.tile([1, 512], F32, tag="rec", name=f"rec_{h}_{qt}", bufs=4)
        nc.vector.reciprocal(rec, cps[HD : HD + 1, :])
        # broadcast 1/denom across 64 partitions on the (idle) GPSIMD
        bc_sb = scratch.tile([64, 512], F32, tag="bcs", name=f"bcs_{h}_{qt}", bufs=3)
        nc.gpsimd.partition_broadcast(bc_sb, rec)
        nc.vector.tensor_mul(
            ctxT_sb[po : po + 64, c, qt * 512 : (qt + 1) * 512],
            cps[0:HD, :],
            bc_sb,
        )

    # ---- pipeline ----------------------------------------------------------
    qk = emit_qk_proj(0)
    prev_E = None
    for c in range(NCH):
        cur_E = [[], []]
        for kb in range(NCH):
            scores_unit(c, kb, qk[0], qk[1], cur_E)
            if c == 0:
                # weave the V projection into iteration 0 (its pv slot is
                # empty) so its DMA waits hide under score matmuls
                emit_v_pair(kb)
            elif kb % 2 == 1:
                # weave previous pair's P@V between score blocks so the PE
                # has ready work while exp() drains score PSUM slots
                g = kb // 2
                pv_unit(c - 1, g // 2, g % 2, prev_E)
        if c + 1 < NCH:
            qk = emit_qk_proj(c + 1)
        prev_E = cur_E
    for g in range(4):
        pv_unit(NCH - 1, g // 2, g % 2, prev_E)

    # ---- out projection: out[q, d_o] = ctx @ Wo.T + bo ----------------------
    for qb in range(NCH):
        for nt in range(2):
            ps = psum.tile([P, 512], F32, tag="ctx", name=f"ops_{qb}_{nt}", bufs=5)
            for c in range(NCH):
                nc.tensor.matmul(
                    ps,
                    lhsT=ctxT_sb[:, c, qb * P : (qb + 1) * P],
                    rhs=wo_sb[:, c, nt * 512 : (nt + 1) * 512],
                    start=(c == 0),
                    stop=(not with_bias) and (c == NCH - 1),
                )
            if with_bias:
                nc.tensor.matmul(
                    ps,
                    lhsT=ones_sb[:, 0:P],
                    rhs=bo_sb[:, nt * 512 : (nt + 1) * 512],
                    start=False,
                    stop=True,
                )
            osb = scratch.tile([P, 512], F32, tag="osb", name=f"osb_{qb}_{nt}")
            nc.scalar.copy(osb, ps)
            nc.sync.dma_start(
                out=out_d[qb * P : (qb + 1) * P, nt * 512 : (nt + 1) * 512], in_=osb
            )


def make_in_maps(hidden_states, causal_attention_mask, Wq, bq, Wk, bk, Wv, bv, Wo, bo):
    f32 = np.float32
    wqT = np.ascontiguousarray((np.asarray(Wq, f32) * SCALE).T).astype(bf16)
    wkT = np.ascontiguousarray(np.asarray(Wk, f32).T).astype(bf16)
    wvT = np.ascontiguousarray(np.asarray(Wv, f32).T).astype(bf16)
    woT = np.ascontiguousarray(np.asarray(Wo, f32).T).astype(bf16)
    bqc = np.ascontiguousarray((np.asarray(bq, f32) * SCALE).reshape(NCH, P).T)
    bkc = np.ascontiguousarray(np.asarray(bk, f32).reshape(NCH, P).T)
    bvr = np.asarray(bv, f32).reshape(1, D).astype(bf16)
    bor = np.asarray(bo, f32).reshape(1, D).astype(bf16)
    # diagonal-block mask, transposed to [k, q], as a 0/1 multiplicative mask
    # (exp(s + m) == exp(s) * [m == 0] for the causal 0/-inf mask)
    mblk = np.asarray(causal_attention_mask, f32)[0, 0, :P, :P]
    maskT = (np.ascontiguousarray(mblk.T) >= 0).astype(bf16)
    shared = {
        "wqT": wqT, "wkT": wkT, "wvT": wvT, "woT": woT,
        "bqc": bqc, "bkc": bkc, "bvr": bvr, "bor": bor, "maskT": maskT,
    }
    hs = np.asarray(hidden_states, f32)
    in_maps = []
    for b in range(B):
        m = dict(shared)
        m["hsT"] = np.ascontiguousarray(hs[b].astype(bf16).T)
        in_maps.append(m)
    return in_maps


_NC_CACHE = {}


def get_nc(with_bias=True):
    if with_bias not in _NC_CACHE:
        _NC_CACHE[with_bias] = build_bass(with_bias=with_bias)
    return _NC_CACHE[with_bias]


def kernel(hidden_states, causal_attention_mask, Wq, bq, Wk, bk, Wv, bv, Wo, bo,
           **run_kwargs):
    with_bias = not (
        np.all(np.asarray(bq) == 0)
        and np.all(np.asarray(bk) == 0)
        and np.all(np.asarray(bv) == 0)
        and np.all(np.asarray(bo) == 0)
    )
    nc = get_nc(with_bias=with_bias)
    in_maps = make_in_maps(
        hidden_states, causal_attention_mask, Wq, bq, Wk, bk, Wv, bv, Wo, bo
    )
    res = bass_utils.run_bass_kernel_spmd(
        nc, in_maps, core_ids=list(range(N_CORES)), **run_kwargs
    )
    out = np.stack([res.results[i]["out"] for i in range(N_CORES)])
    kernel.last_results = res
    return out
